# revision 28
# baseline (speedup 1.0000x reference)
"""Longformer-style sparse attention block (nn_BasicNetwork_22892175688067).

Full-input contract: kernel(**inputs) takes the unsharded inputs and returns
the full [B, S, D] fp32 output.  Internally the (batch, head) grid is sharded
across 8 NeuronCores: core = b*4 + hg owns batch b and heads [3*hg, 3*hg+3).
Each core:
  - gathers x[b] = emb[ids[b]] on-device (bf16 indirect DMA) and transposes
    it with PE identity matmuls (the xbar DMA-transpose path is
    Tile-serialized against SWDGE gathers and stalls the whole front-end),
  - projects q/k (+ global projections) in fp8 DoubleRow (4x PE throughput),
    v in bf16,
  - runs banded window attention + global tokens with packed score PSUM tiles
    (few large exps) and a pipelined Ln/Exp softmax denominator path,
  - applies the output projection slice (bias folded in via a ones row) and
    DMAs PSUM straight to DRAM -> a partial [S, D] fp32 output.
The host sums the 4 per-batch partials (tensor-parallel unshard).

Pipeline layout (engine assignment):
  gpsimd: gathers (front)
  sync:   const loads, recip broadcasts, output stores
  scalar: ids/bv1 loads, qk descales + transpose copies (front), exps (attn)
  vector: fp8 casts + v-bias + transpose copies (front), masks + ctx
          normalize (attention)
  tensor: warmup matmuls (keeps the HAM clock-gate at 2.4 GHz), identity
          transposes, projections, scores, PV, out-proj
"""

import os
import sys
import types

import numpy as np

for _p in ("/opt/trn_rl_repo",):
    if os.path.isdir(_p) and _p not in sys.path:
        sys.path.insert(0, _p)

import ml_dtypes

BF16 = ml_dtypes.bfloat16
F8E4 = ml_dtypes.float8_e4m3

B, S, D = 2, 4096, 768
H, DH, W, G, VOCAB = 12, 64, 256, 16, 50265
HPC = 3               # heads per core
C = S // W            # 16 chunks
QW = W                # queries per chunk (= one-sided window)
P = 128
FT = D // P           # 6 feature k-tiles
TT = S // P           # 32 token tiles
NT = S // 512         # 8 n-tiles for qk projection
NCORES = 8
SCALE = 1.0 / 8.0     # 1/sqrt(DH), exact power of two
X8 = 64.0             # fp8 quantization scale for x and W
DESC_K = 1.0 / (X8 * X8)          # fp8 descale for k-type outputs
DESC_Q = DESC_K * SCALE           # fp8 descale + 1/sqrt(DH) for q-type

# mask block offsets inside the [P, 1536] mask tensor
MO_CBC = 0       # [M_B | M_C]   width 512
MO_CEC = 512     # [M_E | M_C]   width 512
MO_CAD = 1024    # [M_A | M_D]   width 256
MO_CA1D = 1280   # [M_A1 | M_D]  width 256

_COMPILED = {}


def _chunk_slots(c):
    """Score-tile packing for chunk c: a [P, 1024] A-tile + [P, 256] B-tile.

    Returns (a_slots, a_hi, a_mask, b_slots, b_range, b_mask).
    Slots are (jt, col_off); A-slots are 256 wide (queries 0:256),
    B-slots are 128 wide at query offset == col_off.
    Masks are (msk_off, width, col_off).
    """
    if c == 0:
        return ([(2, 0), (4, 256), (3, 512)], 768, (MO_CEC, 512, 0),
                [(5, 128)], (128, 256), (MO_CAD + 128, 128, 128))
    if c == C - 1:
        return ([(1, 0), (2, 256), (3, 512)], 768, (MO_CBC, 256, 0),
                [(0, 0)], (0, 128), (MO_CAD, 128, 0))
    mo_b = MO_CA1D if c == 1 else MO_CAD
    return ([(1, 0), (4, 256), (2, 512), (3, 768)], 1024, (MO_CBC, 512, 0),
            [(0, 0), (5, 128)], (0, 256), (mo_b, 256, 0))


def _build_masks():
    j = np.arange(P)[:, None]
    i = np.arange(QW)[None, :]
    m_a = (j >= i)
    m_a1 = (j >= i) & (j >= G)
    m_b = (P + j >= i)
    m_c = (j <= i)
    m_d = (j <= i - P)
    m_e = (j >= G) & np.ones((1, QW), bool)
    cbc = np.concatenate([m_b, m_c], axis=1)
    cec = np.concatenate([m_e, m_c], axis=1)
    cad = np.concatenate([m_a[:, 0:P], m_d[:, P:QW]], axis=1)
    ca1d = np.concatenate([m_a1[:, 0:P], m_d[:, P:QW]], axis=1)
    msk = np.concatenate([cbc, cec, cad, ca1d], axis=1).astype(np.float32)
    return msk.astype(BF16)                      # [P, 1536]


def _patch_walrus_ldw_opt():
    """Enable the walrus LDWEIGHTS optimization (merges/hides weight loads)."""
    from concourse import bass_utils

    if getattr(bass_utils, "_ldw_opt_patched", False):
        return
    orig = bass_utils.run_command

    def patched(cmd, **kw):
        cmd = [
            "--enable-ldw-opt=true" if c == "--enable-ldw-opt=false" else c
            for c in cmd
        ]
        return orig(cmd, **kw)

    bass_utils.run_command = patched
    bass_utils._ldw_opt_patched = True


def _install_axon_hooks():
    """Provide antenv.axon_hooks (missing in this image) so NTFF tracing works."""
    if "antenv.axon_hooks" in sys.modules:
        return
    mod = types.ModuleType("antenv.axon_hooks")
    hook = [None]
    mod.set_axon_ntff_profile_hook = lambda h: hook.__setitem__(0, h)
    mod.get_axon_ntff_profile_hook = lambda: hook[0]
    sys.modules["antenv.axon_hooks"] = mod
    try:
        import antenv

        antenv.axon_hooks = mod
        from trn_agent_boot.trn_boot import _ntff_profile_via_ctypes

        mod.set_axon_ntff_profile_hook(
            _ntff_profile_via_ctypes("/opt/axon/libaxon_pjrt.so")
        )
    except Exception:
        pass


def _patch_tile_drain():
    """This walrus build allows only ONE sync-wait per instruction.

    Split extra waits onto same-engine NoOps emitted just before the
    instruction (engines execute their stream in order, so chained
    single-wait nops are semantically identical to one multi-wait inst).
    """
    import concourse.mybir as mybir
    import concourse.tile as tile
    from concourse.vector_clock import ScopedClock

    if getattr(tile.TileContext, "_drain_split_patched", False):
        return

    _orig_add = tile.TileContext._add_instruction
    counter = [0]

    def _add_instruction(self, inst):
        si = getattr(inst, "sync_info", None)
        if si is not None and si.on_wait is not None and len(si.on_wait) > 1:
            waits = list(si.on_wait)
            for w in waits[:-1]:
                counter[0] += 1
                nop = mybir.InstNoOp(
                    name=f"WS-{counter[0]}", text_hint="wait_split"
                )
                nop.engine = inst.engine
                nop.sync_info = mybir.SyncInfo(on_wait=[w], on_update=[])
                _orig_add(self, nop)
            inst.sync_info = mybir.SyncInfo(
                on_wait=[waits[-1]], on_update=list(si.on_update)
            )
        _orig_add(self, inst)

    tile.TileContext._add_instruction = _add_instruction

    def _drain_and_barrier(self, tick_clock, wait_clock):
        drain1 = self.nc.sync.drain()
        wait_clock.add_sem_waits(
            drain1.ins, ScopedClock({None: tick_clock.global_clock})
        )
        si = drain1.ins.sync_info
        waits = list(si.on_wait) if si is not None and si.on_wait else []
        if len(waits) > 1:
            drain1.ins.sync_info = mybir.SyncInfo(
                on_wait=waits[:1], on_update=list(si.on_update)
            )
            for wchunk in waits[1:]:
                d = self.nc.sync.drain()
                d.ins.sync_info = mybir.SyncInfo(on_wait=[wchunk], on_update=[])
        self.nc.all_engine_barrier()
        assert self.sems is not None
        popped = self.nc._tile_sem_poison_stack.pop()
        assert popped is self._sem_poison
        self.nc.clear_and_free_semaphores(list(self.sems.allocated().values()))
        self.nc.all_engine_barrier()

    tile.TileContext._drain_and_barrier = _drain_and_barrier
    tile.TileContext._drain_split_patched = True


def build_nc():
    """Build the per-core Bass program (identical on all 8 cores)."""
    import concourse.bass as bass
    import concourse.mybir as mybir
    import concourse.tile as tile

    _patch_tile_drain()

    f32 = mybir.dt.float32
    bf16 = mybir.dt.bfloat16
    f8 = mybir.dt.float8e4
    i32 = mybir.dt.int32
    AF = mybir.ActivationFunctionType
    OP = mybir.AluOpType
    PM = mybir.MatmulPerfMode

    nc = bass.Bass("TRN2", num_devices=NCORES)

    ids = nc.dram_tensor("ids", [P, TT], i32, kind="ExternalInput")
    emb = nc.dram_tensor("emb", [VOCAB, D], bf16, kind="ExternalInput")
    wqk = nc.dram_tensor("wqk", [D, 768], f8, kind="ExternalInput")
    bqk = nc.dram_tensor("bqk", [768], f32, kind="ExternalInput")
    wv = nc.dram_tensor("wv", [D, 384], bf16, kind="ExternalInput")
    bv = nc.dram_tensor("bv", [384], f32, kind="ExternalInput")
    wo0 = nc.dram_tensor("wo0", [P, D], bf16, kind="ExternalInput")
    wo1 = nc.dram_tensor("wo1", [DH + 1, D], bf16, kind="ExternalInput")
    msk = nc.dram_tensor("msk", [P, 1536], bf16, kind="ExternalInput")
    eye = nc.dram_tensor("eye", [P, P], bf16, kind="ExternalInput")
    outp = nc.dram_tensor("out", [S, D], bf16, kind="ExternalOutput")

    with tile.TileContext(nc) as tc:
        from contextlib import ExitStack

        with ExitStack() as ctx:
            pers = ctx.enter_context(tc.tile_pool(name="pers", bufs=1))

            # ---------------- persistent SBUF tensors ----------------
            wqk_sb = pers.tile([P, FT, 768], f8, tag="wqk")
            bqk_sb = pers.tile([P, FT], f32, tag="bqk")
            wv_sb = pers.tile([P, FT, 384], bf16, tag="wv")
            wo0_sb = pers.tile([P, D], bf16, tag="wo0")
            wo1_sb = pers.tile([DH + 1, D], bf16, tag="wo1")
            msk_sb = pers.tile([P, 1536], bf16, tag="msk")
            ones_sb = pers.tile([1, P], f32, tag="ones")
            bv1_sb = pers.tile([1, 384], f32, tag="bv1")
            bvb_sb = pers.tile([P, 384], f32, tag="bvb")
            warm_sb = pers.tile([P, 512], bf16, tag="warm")
            eye_sb = pers.tile([P, P], bf16, tag="eye")
            # projection outputs
            blk_sb = pers.tile([P, 5, S], bf16, tag="blk")
            qg5_sb = pers.tile([P, P], bf16, tag="qg5")
            v_sb = pers.tile([P, TT, 2 * HPC, DH + 1], bf16, tag="vsb")
            vg_sb = pers.tile([3 * 2 * G, DH + 1], bf16, tag="vg")
            ctx01_sb = pers.tile([P, S], bf16, tag="ctx01")
            ctx2_sb = pers.tile([DH + 1, S], bf16, tag="ctx2")
            ctxh1_sb = pers.tile([DH, S], bf16, tag="ctxh1")
            pg_sb = pers.tile([P, TT, HPC * G], bf16, tag="pgsb")

            # ---------------- constant loads ----------------
            # token ids + bv go FIRST, on the scalar queue, so the gathers
            # and the bvb broadcast are not stuck behind the weight loads
            ids_sb = pers.tile([P, TT], i32, tag="ids")
            nc.scalar.dma_start(out=ids_sb[:], in_=ids.ap())
            nc.scalar.dma_start(out=bv1_sb[:], in_=bv.ap()[None, :])
            nc.scalar.dma_start(out=eye_sb[:], in_=eye.ap())
            nc.sync.dma_start(
                out=wqk_sb[:], in_=wqk.ap().rearrange("(kt p) c -> p kt c", p=P)
            )
            nc.sync.dma_start(
                out=bqk_sb[:], in_=bqk.ap().rearrange("(kt p) -> p kt", p=P)
            )
            nc.sync.dma_start(
                out=wv_sb[:], in_=wv.ap().rearrange("(kt p) c -> p kt c", p=P)
            )
            nc.sync.dma_start(out=wo0_sb[:], in_=wo0.ap())
            nc.sync.dma_start(out=wo1_sb[:], in_=wo1.ap())
            nc.sync.dma_start(out=msk_sb[:], in_=msk.ap())
            c1_sb = pers.tile([1, 2 * QW], f32, tag="c1t")
            nc.gpsimd.memset(warm_sb[:], 0.0)
            nc.gpsimd.memset(c1_sb[:], 2.0017324)
            nc.gpsimd.memset(ones_sb[:], 1.0)
            nc.gpsimd.memset(v_sb[:, :, :, DH], 1.0)
            nc.gpsimd.memset(ctx2_sb[DH : DH + 1, :], 1.0)

            # PE warmup: ~17us of dependency-free matmuls so the HAM clock
            # gate reaches 8/8 (2.4 GHz) before the first real matmul, and
            # stays there through the gather ramp.
            with tc.tile_pool(name="warm_ps", bufs=1, space="PSUM") as wps:
                wpt = wps.tile([P, 512], f32, tag="wp")
                NWARM = 40
                for i in range(NWARM):
                    nc.tensor.matmul(
                        out=wpt[:], lhsT=warm_sb[:, 0:P], rhs=warm_sb[:],
                        start=(i == 0), stop=(i == NWARM - 1),
                    )

            # broadcast v-bias across partitions via ones-matmul (one-time)
            with tc.tile_pool(name="bcast_ps", bufs=1, space="PSUM") as bps:
                bvp = bps.tile([P, 384], f32, tag="bvp")
                nc.tensor.matmul(
                    out=bvp[:], lhsT=ones_sb[0:1, :], rhs=bv1_sb[:],
                    start=True, stop=True,
                )
                nc.vector.tensor_copy(out=bvb_sb[:], in_=bvp[:])

            # ---------------- gather + xbar transpose + projections --------
            with tc.tile_pool(name="gather", bufs=12) as gp, \
                 tc.tile_pool(name="xtp", bufs=3) as xp, \
                 tc.tile_pool(name="tp_ps", bufs=2, space="PSUM") as tps, \
                 tc.tile_pool(name="proj_ps", bufs=6, space="PSUM") as pps:

                xT_t = [None] * NT
                x8_t = [None] * NT

                def alloc_group(n):
                    xT_t[n] = xp.tile([P, FT, 512], bf16, tag="xT", name=f"xT{n}")
                    x8_t[n] = xp.tile([P, FT, 512], f8, tag="x8", name=f"x8{n}")

                xg_t = {}

                def emit_gather(tt):
                    xg = gp.tile([P, D], bf16, tag="xg")
                    nc.gpsimd.indirect_dma_start(
                        out=xg[:],
                        out_offset=None,
                        in_=emb.ap(),
                        in_offset=bass.IndirectOffsetOnAxis(
                            ap=ids_sb[:, tt : tt + 1], axis=0
                        ),
                    )
                    xg_t[tt] = xg

                def emit_transpose(tt):
                    # PE transpose: [tok, feat] -> [feat%P, ft, tok].  (The
                    # xbar dma_start_transpose path is Tile-serialized against
                    # every SWDGE gather DMA — a ~2us handoff per link — so
                    # identity matmuls + a PSUM copy are much faster here.)
                    n, j = divmod(tt, 4)
                    xg = xg_t.pop(tt)
                    tp = tps.tile([P, FT * P], bf16, tag="tp")
                    for ft in range(FT):
                        nc.tensor.transpose(
                            out=tp[:, ft * P : (ft + 1) * P],
                            in_=xg[:, ft * P : (ft + 1) * P],
                            identity=eye_sb[:],
                        )
                    dst = xT_t[n][:, :, j * P : (j + 1) * P]
                    if tt % 2 == 0:
                        nc.vector.tensor_copy(out=dst, in_=tp[:])
                    else:
                        nc.scalar.copy(out=dst, in_=tp[:])

                def emit_cast(n):
                    # fp8 copy (scaled) for the q/k projections, whole group
                    nc.vector.tensor_scalar_mul(
                        out=x8_t[n][:], in0=xT_t[n][:], scalar1=X8
                    )

                # q/k/qg/kg projections in fp8 DoubleRow (feature-major out).
                # block cols: 0:(q0|q1) 1:(k0|k1) 2:(q2|qg2) 3:(k2|kg2)
                #             4:(kg0|kg1); B5 (qg0|qg1) over token-tile 0 only
                desc = [DESC_Q, DESC_K, DESC_Q, DESC_K, DESC_K, DESC_Q]
                for n in range(3):
                    alloc_group(n)
                for tt in range(12):
                    emit_gather(tt)
                for tt in range(4):
                    emit_transpose(tt)
                emit_cast(0)
                for n in range(NT):
                    if n + 1 < NT:
                        for tt in range(4 * n + 4, 4 * n + 8):
                            emit_transpose(tt)
                        emit_cast(n + 1)
                    for bi in range(5):
                        ps = pps.tile([P, 512], f32, tag="pps")
                        for kt in range(FT // 2):
                            nc.tensor.matmul(
                                out=ps[:],
                                lhsT=wqk_sb[:, 2 * kt : 2 * kt + 2,
                                            bi * P : (bi + 1) * P],
                                rhs=x8_t[n][:, 2 * kt : 2 * kt + 2, :],
                                start=(kt == 0),
                                stop=(kt == FT // 2 - 1),
                                perf_mode=PM.DoubleRow,
                            )
                        nc.scalar.activation(
                            blk_sb[:, bi, n * 512 : (n + 1) * 512],
                            ps[:],
                            AF.Identity,
                            bias=bqk_sb[:, bi : bi + 1],
                            scale=desc[bi],
                        )
                    if n == 0:
                        # B5 (qg0|qg1) over token-tile 0 — must run while
                        # x8_t[0]'s ring slot is still live
                        ps5 = pps.tile([P, 512], f32, tag="pps")
                        for kt in range(FT // 2):
                            nc.tensor.matmul(
                                out=ps5[:, 0:P],
                                lhsT=wqk_sb[:, 2 * kt : 2 * kt + 2,
                                            5 * P : 6 * P],
                                rhs=x8_t[0][:, 2 * kt : 2 * kt + 2, 0:P],
                                start=(kt == 0),
                                stop=(kt == FT // 2 - 1),
                                perf_mode=PM.DoubleRow,
                            )
                        nc.scalar.activation(
                            qg5_sb[:], ps5[:, 0:P], AF.Identity,
                            bias=bqk_sb[:, 5:6], scale=desc[5],
                        )
                    # v/vg projections for the same token tiles (token-major)
                    for j in range(4):
                        tt = 4 * n + j
                        vp = pps.tile([P, 512], f32, tag="pps")
                        for kt in range(FT):
                            nc.tensor.matmul(
                                out=vp[:, 0:384],
                                lhsT=xT_t[n][:, kt, j * P : (j + 1) * P],
                                rhs=wv_sb[:, kt, :],
                                start=(kt == 0),
                                stop=(kt == FT - 1),
                            )
                        nc.vector.tensor_tensor(
                            out=v_sb[:, tt, :, 0:DH],
                            in0=vp[:, 0:384],
                            in1=bvb_sb[:],
                            op=OP.add,
                        )
                    # prefetch group n+3's gathers (after group n's readers
                    # are emitted, so the xT/x8 ring reuse deps are correct)
                    if n + 3 < NT:
                        alloc_group(n + 3)
                        for tt in range(4 * n + 12, 4 * n + 16):
                            emit_gather(tt)
                # global-token v rows (first G tokens), per head, incl ones
                # col, placed at partitions 32h to match the gk score tiles
                for h in range(HPC):
                    nc.sync.dma_start(
                        out=vg_sb[32 * h : 32 * h + G, :],
                        in_=v_sb[0:G, 0, h, :],
                    )

            # operand views (each matmul operand pair shares a base partition)
            qv = [blk_sb[0:DH, 0, :], blk_sb[DH:P, 0, :], blk_sb[0:DH, 2, :]]
            kv = [blk_sb[0:DH, 1, :], blk_sb[DH:P, 1, :], blk_sb[0:DH, 3, :]]
            qgv = [qg5_sb[0:DH, 0:G], qg5_sb[DH:P, 0:G], blk_sb[DH:P, 2, 0:G]]
            kgv = [blk_sb[0:DH, 4, :], blk_sb[DH:P, 4, :], blk_sb[DH:P, 3, :]]
            ctxdst = [ctx01_sb[0:DH, :], ctxh1_sb[:, :], ctx2_sb[0:DH, :]]

            # ---------------- attention ----------------
            with tc.tile_pool(name="pa_sb", bufs=4) as pap, \
                 tc.tile_pool(name="pb_sb", bufs=3) as pbp, \
                 tc.tile_pool(name="pgk_sb", bufs=3) as pgkp, \
                 tc.tile_pool(name="rec_sb", bufs=3) as recp, \
                 tc.tile_pool(name="sa_ps", bufs=2, space="PSUM") as spa, \
                 tc.tile_pool(name="ctx_ps", bufs=2, space="PSUM") as cps:
                from contextlib import ExitStack as _ES

                es_bg = _ES()
                spb = es_bg.enter_context(
                    tc.tile_pool(name="sb_ps", bufs=1, space="PSUM")
                )
                spg = es_bg.enter_context(
                    tc.tile_pool(name="gk_ps", bufs=1, space="PSUM")
                )

                gkf = spg.tile([3 * 2 * G, 2 * QW], f32, tag="gk")
                gk_ps = gkf[:]
                nc.vector.memset(gk_ps[:], 0.0)

                def emit_filler():
                    pass

                def emit_pv(h, c, pgk_t, ptiles, cpsum):
                    """PV accumulation for chunk c into its half of cpsum."""
                    cq = (c % 2) * QW
                    nc.tensor.matmul(
                        out=cpsum[:, cq : cq + QW],
                        lhsT=vg_sb[32 * h : 32 * h + G, :],
                        rhs=pgk_t[32 * h : 32 * h + G, cq : cq + QW],
                        start=True,
                        stop=False,
                        tile_position=(32 * h, 0),
                    )
                    for idx, (g_tt, p_ap, x0, x1) in enumerate(ptiles):
                        nc.tensor.matmul(
                            out=cpsum[:, cq + x0 : cq + x1],
                            lhsT=v_sb[:, g_tt, h, :],
                            rhs=p_ap,
                            start=False,
                            stop=(idx == len(ptiles) - 1),
                        )

                def emit_norm(st):
                    h, cp, cpsum, recb = st
                    nc.vector.tensor_tensor(
                        out=ctxdst[h][:, cp * 2 * QW : (cp + 1) * 2 * QW],
                        in0=cpsum[0:DH, :],
                        in1=recb[:],
                        op=OP.mult,
                    )

                def emit_scores(h, c, psb, cb):
                    a_slots, a_hi, a_mask, b_slots, b_rng, b_mask = \
                        _chunk_slots(c)
                    q0 = c * QW
                    psa = spa.tile([P, 1024], f32, tag="psa")
                    for jt, so in a_slots:
                        tok0 = (c - 1) * 2 * P + jt * P
                        nc.tensor.matmul(
                            out=psa[:, so : so + QW],
                            lhsT=kv[h][:, tok0 : tok0 + P],
                            rhs=qv[h][:, q0 : q0 + QW],
                            start=True, stop=True,
                        )
                    for jt, so in b_slots:
                        tok0 = (c - 1) * 2 * P + jt * P
                        nc.tensor.matmul(
                            out=psb[:, cb + so : cb + so + P],
                            lhsT=kv[h][:, tok0 : tok0 + P],
                            rhs=qv[h][:, q0 + so : q0 + so + P],
                            start=True, stop=True,
                        )
                    pA = pap.tile([P, 1024], bf16, tag="pa")
                    nc.scalar.activation(pA[:, 0:a_hi], psa[:, 0:a_hi], AF.Exp)
                    pB = pbp.tile([P, QW], bf16, tag="pb")
                    b0, b1 = b_rng
                    nc.scalar.activation(
                        pB[:, b0:b1], psb[:, cb + b0 : cb + b1], AF.Exp
                    )
                    for mo, mw, mso in (a_mask,):
                        nc.vector.tensor_tensor(
                            out=pA[:, mso : mso + mw],
                            in0=pA[:, mso : mso + mw],
                            in1=msk_sb[:, mo : mo + mw],
                            op=OP.mult,
                        )
                    mo, mw, mso = b_mask
                    nc.vector.tensor_tensor(
                        out=pB[:, mso : mso + mw],
                        in0=pB[:, mso : mso + mw],
                        in1=msk_sb[:, mo : mo + mw],
                        op=OP.mult,
                    )
                    ptiles = [
                        (2 * (c - 1) + jt, pA[:, so : so + QW], 0, QW)
                        for jt, so in a_slots
                    ] + [
                        (2 * (c - 1) + jt, pB[:, so : so + P], so, so + P)
                        for jt, so in b_slots
                    ]
                    return ptiles

                def emit_global_rows(h):
                    # global query rows for head h; independent of the
                    # windowed chain, interleaved to keep the PE duty high
                    gp_ps = spa.tile([P, 1024], f32, tag="psa")
                    for tt in range(TT):
                        nc.tensor.matmul(
                            out=gp_ps[:, tt * G : (tt + 1) * G],
                            lhsT=kgv[h][:, tt * P : (tt + 1) * P],
                            rhs=qgv[h][:],
                            start=True,
                            stop=True,
                        )
                    nc.scalar.activation(
                        pg_sb[:, :, h * G : (h + 1) * G],
                        gp_ps[:, 0 : TT * G],
                        AF.Exp,
                    )
                    gc_ps = cps.tile([DH + 1, 2 * QW], f32, tag="cps")
                    for tt in range(TT):
                        nc.tensor.matmul(
                            out=gc_ps[:, 0:G],
                            lhsT=v_sb[:, tt, HPC + h, :],
                            rhs=pg_sb[:, tt, h * G : (h + 1) * G],
                            start=(tt == 0),
                            stop=(tt == TT - 1),
                        )
                    deng = recp.tile([1, 2 * QW], f32, tag="den", name="dg")
                    nc.scalar.activation(
                        deng[:, 0:G], gc_ps[DH : DH + 1, 0:G], AF.Ln
                    )
                    recg = recp.tile([1, 2 * QW], bf16, tag="rec")
                    nc.scalar.activation(
                        recg[:, 0:G], deng[:, 0:G], AF.Exp, scale=-1.0
                    )
                    recgb = recp.tile([DH, 2 * QW], bf16, tag="recb")
                    nc.sync.dma_start(
                        out=recgb[:, None, 0:G],
                        in_=recg[0:1, None, 0:G].to_broadcast([1, DH, G]),
                    )
                    nc.vector.tensor_tensor(
                        out=ctxdst[h][:, 0:G],
                        in0=gc_ps[0:DH, 0:G],
                        in1=recgb[:, 0:G],
                        op=OP.mult,
                    )

                from collections import deque

                pending = deque()    # (h, c, pgk, ptiles, cpsum); PV lag 2

                def emit_recip(h, cp, cpsum):
                    """Paired-chunk denominator -> 1/x -> broadcast DMA."""
                    ln_t = recp.tile([1, 2 * QW], f32, tag="den", name="lnt")
                    nc.scalar.activation(
                        ln_t[:], cpsum[DH : DH + 1, :], AF.Ln
                    )
                    rec = recp.tile([1, 2 * QW], bf16, tag="rec")
                    nc.scalar.activation(rec[:], ln_t[:], AF.Exp, scale=-1.0)
                    recb = recp.tile([DH, 2 * QW], bf16, tag="recb")
                    nc.sync.dma_start(
                        out=recb[:, None, :],
                        in_=rec[0:1, None, :].to_broadcast([1, DH, 2 * QW]),
                    )
                    return (h, cp, cpsum, recb)

                norm_box = [None]

                def pop_pv():
                    ent = pending.popleft()
                    emit_pv(*ent)
                    emit_filler()
                    ph, pc = ent[0], ent[1]
                    if pc % 2 == 1:
                        st = emit_recip(ph, pc // 2, ent[4])
                        if norm_box[0] is not None:
                            emit_norm(norm_box[0])
                        norm_box[0] = st

                def emit_gk(cp):
                    # global-key scores + exp for pair cp (prefetched one
                    # pair early so the exp isn't part of the pair-boundary
                    # ACT bundle that stalls the PE)
                    q0 = cp * 2 * QW
                    for h in range(HPC):
                        nc.tensor.matmul(
                            out=gk_ps[32 * h : 32 * h + G, :],
                            lhsT=kv[h][:, 0:G],
                            rhs=qv[h][:, q0 : q0 + 2 * QW],
                            start=True, stop=True,
                            tile_position=(0 if h != 1 else DH, 32 * h),
                        )
                    pgk_t = pgkp.tile([3 * 2 * G, 2 * QW], bf16, tag="pgk")
                    nc.scalar.activation(pgk_t[:], gk_ps[:], AF.Exp)
                    return pgk_t

                pgk_next = emit_gk(0)
                for cp in range(C // 2):
                    pgk_t = pgk_next
                    for h in range(HPC):
                        cpsum = cps.tile([DH + 1, 2 * QW], f32, tag="cps")
                        psb = spb.tile([P, 2 * QW], f32, tag="psb")
                        for ci, c in enumerate((2 * cp, 2 * cp + 1)):
                            ptiles = emit_scores(h, c, psb, ci * QW)
                            emit_filler()
                            pending.append((h, c, pgk_t, ptiles, cpsum))
                            if len(pending) > 2:
                                pop_pv()
                        if h == 0 and cp + 1 < C // 2:
                            pgk_next = emit_gk(cp + 1)
                while pending:
                    pop_pv()
                emit_norm(norm_box[0])
                es_bg.close()    # free the spb/gk banks for the out-proj

                with tc.tile_pool(name="out_sb", bufs=4) as osb, \
                     tc.tile_pool(name="out_ps", bufs=2, space="PSUM") as ops:

                    def emit_outproj(tt):
                        # two [P, 384] half-psums (1 bank each, ring of 2)
                        opa = ops.tile([P, 384], f32, tag="oph", name="opa")
                        opb = ops.tile([P, 384], f32, tag="oph", name="opb")
                        # group by lhsT so the PE loads each ctx tile once
                        for half, n0 in ((opa, 0), (opb, 384)):
                            nc.tensor.matmul(
                                out=half[:],
                                lhsT=ctx01_sb[:, tt * P : (tt + 1) * P],
                                rhs=wo0_sb[:, n0 : n0 + 384],
                                start=True,
                                stop=False,
                            )
                        for half, n0 in ((opa, 0), (opb, 384)):
                            nc.tensor.matmul(
                                out=half[:],
                                lhsT=ctx2_sb[:, tt * P : (tt + 1) * P],
                                rhs=wo1_sb[:, n0 : n0 + 384],
                                start=False,
                                stop=True,
                            )
                        ot = osb.tile([P, D], bf16, tag="ot")
                        nc.vector.tensor_copy(out=ot[:, 0:384], in_=opa[:])
                        nc.scalar.copy(out=ot[:, 384:768], in_=opb[:])
                        nc.sync.dma_start(
                            out=outp.ap()[tt * P : (tt + 1) * P, :], in_=ot[:]
                        )

                    # head 1 ctx (tokens P:S) -> ctx01 rows 64:128; the 0:P
                    # slice follows after global h1 overwrites its gctx cols
                    nc.sync.dma_start(
                        out=ctx01_sb[DH:P, P:S], in_=ctxh1_sb[:, P:S]
                    )
                    emit_global_rows(0)
                    for tt in range(1, 12):
                        emit_outproj(tt)
                    emit_global_rows(1)
                    nc.sync.dma_start(
                        out=ctx01_sb[DH:P, 0:P], in_=ctxh1_sb[:, 0:P]
                    )
                    for tt in range(12, 22):
                        emit_outproj(tt)
                    emit_global_rows(2)
                    for tt in range(22, TT):
                        emit_outproj(tt)
                    emit_outproj(0)

    # populate .instr bytes for extended-inst InstISA subclasses (the custom
    # DVE reciprocal) — raw Bass skips this pass and the NEFF compiler then
    # fails with "ISA wrong length"
    from concourse import library_overlay

    library_overlay.lower_extended_insts(nc)
    return nc


def _prep_core_inputs(core, input_ids, emb, Wq, bq, Wk, bk, Wv, bv,
                      Wqg, bqg, Wkg, bkg, Wvg, bvg, Wo, bo, _emb_bf=None):
    b, hg = divmod(core, 4)
    hs = HPC * hg * DH           # feature offset of this core's head slice
    sl = slice(hs, hs + HPC * DH)

    def hcol(Wm, h):
        return np.asarray(Wm[:, hs + h * DH : hs + (h + 1) * DH], np.float32)

    def hbias(bm, h):
        return np.asarray(bm[hs + h * DH : hs + (h + 1) * DH], np.float32)

    # blocks: 0:(q0|q1) 1:(k0|k1) 2:(q2|qg2) 3:(k2|kg2) 4:(kg0|kg1) 5:(qg0|qg1)
    wq = [hcol(Wq, h) for h in range(HPC)]
    wk = [hcol(Wk, h) for h in range(HPC)]
    wqg = [hcol(Wqg, h) for h in range(HPC)]
    wkg = [hcol(Wkg, h) for h in range(HPC)]
    bq_ = [hbias(bq, h) * SCALE for h in range(HPC)]
    bk_ = [hbias(bk, h) for h in range(HPC)]
    bqg_ = [hbias(bqg, h) * SCALE for h in range(HPC)]
    bkg_ = [hbias(bkg, h) for h in range(HPC)]

    wqk_cat = np.concatenate(
        [wq[0], wq[1], wk[0], wk[1], wq[2], wqg[2], wk[2], wkg[2],
         wkg[0], wkg[1], wqg[0], wqg[1]], axis=1) * X8
    bqk_cat = np.concatenate(
        [bq_[0], bq_[1], bk_[0], bk_[1], bq_[2], bqg_[2], bk_[2], bkg_[2],
         bkg_[0], bkg_[1], bqg_[0], bqg_[1]])

    wv_cat = np.concatenate(
        [hcol(Wv, h) for h in range(HPC)] + [hcol(Wvg, h) for h in range(HPC)],
        axis=1)
    bv_cat = np.concatenate(
        [hbias(bv, h) for h in range(HPC)] + [hbias(bvg, h) for h in range(HPC)])

    wo_cat = np.asarray(Wo[sl, :], np.float32)
    bo_row = (np.asarray(bo, np.float32) if hg == 0
              else np.zeros((D,), np.float32))
    wo1_cat = np.concatenate([wo_cat[P : P + DH, :], bo_row[None, :]], axis=0)

    if _emb_bf is None:
        _emb_bf = np.ascontiguousarray(np.asarray(emb, np.float32)).astype(BF16)

    return {
        "ids": np.ascontiguousarray(
            np.asarray(input_ids[b], np.int32).reshape(TT, P).T),
        "emb": _emb_bf,
        "wqk": wqk_cat.astype(F8E4),
        "bqk": bqk_cat.astype(np.float32),
        "wv": wv_cat.astype(BF16),
        "bv": bv_cat.astype(np.float32),
        "wo0": np.ascontiguousarray(wo_cat[0:P, :]).astype(BF16),
        "wo1": np.ascontiguousarray(wo1_cat).astype(BF16),
        "msk": _build_masks(),
        "eye": np.eye(P, dtype=np.float32).astype(BF16),
    }


def kernel(**inputs):
    _install_axon_hooks()
    from concourse.bass_utils import run_bass_kernel_spmd

    if "nc" not in _COMPILED:
        _COMPILED["nc"] = build_nc()
    nc = _COMPILED["nc"]

    emb_bf = np.ascontiguousarray(
        np.asarray(inputs["emb"], np.float32)).astype(BF16)
    in_maps = [
        _prep_core_inputs(core, _emb_bf=emb_bf, **inputs)
        for core in range(NCORES)
    ]
    trace = bool(int(os.environ.get("KERNEL_TRACE", "0")))
    res = run_bass_kernel_spmd(nc, in_maps, list(range(NCORES)), trace=trace)
    _COMPILED["last_result"] = res

    out = np.zeros((B, S, D), np.float32)
    for core in range(NCORES):
        out[core // 4] += np.asarray(res.results[core]["out"], np.float32)
    return out


# revision 30
# speedup vs baseline: 1.0842x; 1.0842x over previous
"""Longformer-style sparse attention block (nn_BasicNetwork_22892175688067).

Full-input contract: kernel(**inputs) takes the unsharded inputs and returns
the full [B, S, D] fp32 output.  Internally the (batch, head) grid is sharded
across 8 NeuronCores: core = b*4 + hg owns batch b and heads [3*hg, 3*hg+3).
Each core:
  - gathers x[b] = emb[ids[b]] on-device (bf16 indirect DMA) and transposes
    it with PE identity matmuls (the xbar DMA-transpose path is
    Tile-serialized against SWDGE gathers and stalls the whole front-end),
  - projects q/k (+ global projections) in fp8 DoubleRow (4x PE throughput),
    v in bf16,
  - runs banded window attention + global tokens with packed score PSUM tiles
    (few large exps) and a pipelined Ln/Exp softmax denominator path,
  - applies the output projection slice (bias folded in via a ones row) and
    DMAs PSUM straight to DRAM -> a partial [S, D] fp32 output.
The host sums the 4 per-batch partials (tensor-parallel unshard).

Pipeline layout (engine assignment):
  gpsimd: gathers (front)
  sync:   const loads, recip broadcasts, output stores
  scalar: ids/bv1 loads, qk descales + transpose copies (front), exps (attn)
  vector: fp8 casts + v-bias + transpose copies (front), masks + ctx
          normalize (attention)
  tensor: warmup matmuls (keeps the HAM clock-gate at 2.4 GHz), identity
          transposes, projections, scores, PV, out-proj
"""

import os
import sys
import types

import numpy as np

for _p in ("/opt/trn_rl_repo",):
    if os.path.isdir(_p) and _p not in sys.path:
        sys.path.insert(0, _p)

import ml_dtypes

BF16 = ml_dtypes.bfloat16
F8E4 = ml_dtypes.float8_e4m3

B, S, D = 2, 4096, 768
H, DH, W, G, VOCAB = 12, 64, 256, 16, 50265
HPC = 3               # heads per core
C = S // W            # 16 chunks
QW = W                # queries per chunk (= one-sided window)
P = 128
FT = D // P           # 6 feature k-tiles
TT = S // P           # 32 token tiles
NT = S // 512         # 8 n-tiles for qk projection
NCORES = 8
SCALE = 1.0 / 8.0     # 1/sqrt(DH), exact power of two
X8 = 64.0             # fp8 quantization scale for x and W
DESC_K = 1.0 / (X8 * X8)          # fp8 descale for k-type outputs
DESC_Q = DESC_K * SCALE           # fp8 descale + 1/sqrt(DH) for q-type

# mask block offsets inside the [P, 1536] mask tensor
MO_CBC = 0       # [M_B | M_C]   width 512
MO_CEC = 512     # [M_E | M_C]   width 512
MO_CAD = 1024    # [M_A | M_D]   width 256
MO_CA1D = 1280   # [M_A1 | M_D]  width 256

_COMPILED = {}


def _chunk_slots(c):
    """Score-tile packing for chunk c: a [P, 1024] A-tile + [P, 256] B-tile.

    Returns (a_slots, a_hi, a_mask, b_slots, b_range, b_mask).
    Slots are (jt, col_off); A-slots are 256 wide (queries 0:256),
    B-slots are 128 wide at query offset == col_off.
    Masks are (msk_off, width, col_off).
    """
    if c == 0:
        return ([(2, 0), (4, 256), (3, 512)], 768, (MO_CEC, 512, 0),
                [(5, 128)], (128, 256), (MO_CAD + 128, 128, 128))
    if c == C - 1:
        return ([(1, 0), (2, 256), (3, 512)], 768, (MO_CBC, 256, 0),
                [(0, 0)], (0, 128), (MO_CAD, 128, 0))
    mo_b = MO_CA1D if c == 1 else MO_CAD
    return ([(1, 0), (4, 256), (2, 512), (3, 768)], 1024, (MO_CBC, 512, 0),
            [(0, 0), (5, 128)], (0, 256), (mo_b, 256, 0))


def _build_masks():
    j = np.arange(P)[:, None]
    i = np.arange(QW)[None, :]
    m_a = (j >= i)
    m_a1 = (j >= i) & (j >= G)
    m_b = (P + j >= i)
    m_c = (j <= i)
    m_d = (j <= i - P)
    m_e = (j >= G) & np.ones((1, QW), bool)
    cbc = np.concatenate([m_b, m_c], axis=1)
    cec = np.concatenate([m_e, m_c], axis=1)
    cad = np.concatenate([m_a[:, 0:P], m_d[:, P:QW]], axis=1)
    ca1d = np.concatenate([m_a1[:, 0:P], m_d[:, P:QW]], axis=1)
    msk = np.concatenate([cbc, cec, cad, ca1d], axis=1).astype(np.float32)
    return msk.astype(BF16)                      # [P, 1536]


def _patch_walrus_ldw_opt():
    """Enable the walrus LDWEIGHTS optimization (merges/hides weight loads)."""
    from concourse import bass_utils

    if getattr(bass_utils, "_ldw_opt_patched", False):
        return
    orig = bass_utils.run_command

    def patched(cmd, **kw):
        cmd = [
            "--enable-ldw-opt=true" if c == "--enable-ldw-opt=false" else c
            for c in cmd
        ]
        return orig(cmd, **kw)

    bass_utils.run_command = patched
    bass_utils._ldw_opt_patched = True


def _install_axon_hooks():
    """Provide antenv.axon_hooks (missing in this image) so NTFF tracing works."""
    if "antenv.axon_hooks" in sys.modules:
        return
    mod = types.ModuleType("antenv.axon_hooks")
    hook = [None]
    mod.set_axon_ntff_profile_hook = lambda h: hook.__setitem__(0, h)
    mod.get_axon_ntff_profile_hook = lambda: hook[0]
    sys.modules["antenv.axon_hooks"] = mod
    try:
        import antenv

        antenv.axon_hooks = mod
        from trn_agent_boot.trn_boot import _ntff_profile_via_ctypes

        mod.set_axon_ntff_profile_hook(
            _ntff_profile_via_ctypes("/opt/axon/libaxon_pjrt.so")
        )
    except Exception:
        pass


def _patch_tile_drain():
    """This walrus build allows only ONE sync-wait per instruction.

    Split extra waits onto same-engine NoOps emitted just before the
    instruction (engines execute their stream in order, so chained
    single-wait nops are semantically identical to one multi-wait inst).
    """
    import concourse.mybir as mybir
    import concourse.tile as tile
    from concourse.vector_clock import ScopedClock

    if getattr(tile.TileContext, "_drain_split_patched", False):
        return

    _orig_add = tile.TileContext._add_instruction
    counter = [0]

    def _add_instruction(self, inst):
        si = getattr(inst, "sync_info", None)
        if si is not None and si.on_wait is not None and len(si.on_wait) > 1:
            waits = list(si.on_wait)
            for w in waits[:-1]:
                counter[0] += 1
                nop = mybir.InstNoOp(
                    name=f"WS-{counter[0]}", text_hint="wait_split"
                )
                nop.engine = inst.engine
                nop.sync_info = mybir.SyncInfo(on_wait=[w], on_update=[])
                _orig_add(self, nop)
            inst.sync_info = mybir.SyncInfo(
                on_wait=[waits[-1]], on_update=list(si.on_update)
            )
        _orig_add(self, inst)

    tile.TileContext._add_instruction = _add_instruction

    def _drain_and_barrier(self, tick_clock, wait_clock):
        drain1 = self.nc.sync.drain()
        wait_clock.add_sem_waits(
            drain1.ins, ScopedClock({None: tick_clock.global_clock})
        )
        si = drain1.ins.sync_info
        waits = list(si.on_wait) if si is not None and si.on_wait else []
        if len(waits) > 1:
            drain1.ins.sync_info = mybir.SyncInfo(
                on_wait=waits[:1], on_update=list(si.on_update)
            )
            for wchunk in waits[1:]:
                d = self.nc.sync.drain()
                d.ins.sync_info = mybir.SyncInfo(on_wait=[wchunk], on_update=[])
        self.nc.all_engine_barrier()
        assert self.sems is not None
        popped = self.nc._tile_sem_poison_stack.pop()
        assert popped is self._sem_poison
        self.nc.clear_and_free_semaphores(list(self.sems.allocated().values()))
        self.nc.all_engine_barrier()

    tile.TileContext._drain_and_barrier = _drain_and_barrier
    tile.TileContext._drain_split_patched = True


def build_nc():
    """Build the per-core Bass program (identical on all 8 cores)."""
    import concourse.bass as bass
    import concourse.mybir as mybir
    import concourse.tile as tile

    _patch_tile_drain()

    f32 = mybir.dt.float32
    bf16 = mybir.dt.bfloat16
    f8 = mybir.dt.float8e4
    i32 = mybir.dt.int32
    AF = mybir.ActivationFunctionType
    OP = mybir.AluOpType
    PM = mybir.MatmulPerfMode

    nc = bass.Bass("TRN2", num_devices=NCORES)

    ids = nc.dram_tensor("ids", [P, TT], i32, kind="ExternalInput")
    emb = nc.dram_tensor("emb", [VOCAB, D], bf16, kind="ExternalInput")
    wqk = nc.dram_tensor("wqk", [D, 768], f8, kind="ExternalInput")
    bqk = nc.dram_tensor("bqk", [768], f32, kind="ExternalInput")
    wv = nc.dram_tensor("wv", [D, 384], bf16, kind="ExternalInput")
    bv = nc.dram_tensor("bv", [384], f32, kind="ExternalInput")
    wo0 = nc.dram_tensor("wo0", [P, D], bf16, kind="ExternalInput")
    wo1 = nc.dram_tensor("wo1", [DH + 1, D], bf16, kind="ExternalInput")
    msk = nc.dram_tensor("msk", [P, 1536], bf16, kind="ExternalInput")
    eye = nc.dram_tensor("eye", [P, P], bf16, kind="ExternalInput")
    outp = nc.dram_tensor("out", [S, D], bf16, kind="ExternalOutput")

    with tile.TileContext(nc) as tc:
        from contextlib import ExitStack

        with ExitStack() as ctx:
            pers = ctx.enter_context(tc.tile_pool(name="pers", bufs=1))

            # ---------------- persistent SBUF tensors ----------------
            wqk_sb = pers.tile([P, FT, 768], f8, tag="wqk")
            bqk_sb = pers.tile([P, FT], f32, tag="bqk")
            wv_sb = pers.tile([P, FT, 384], bf16, tag="wv")
            wo0_sb = pers.tile([P, D], bf16, tag="wo0")
            wo1_sb = pers.tile([DH + 1, D], bf16, tag="wo1")
            msk_sb = pers.tile([P, 1536], bf16, tag="msk")
            ones_sb = pers.tile([1, P], f32, tag="ones")
            bv1_sb = pers.tile([1, 384], f32, tag="bv1")
            bvb_sb = pers.tile([P, 384], f32, tag="bvb")
            warm_sb = pers.tile([P, 512], bf16, tag="warm")
            eye_sb = pers.tile([P, P], bf16, tag="eye")
            # projection outputs
            blk_sb = pers.tile([P, 5, S], bf16, tag="blk")
            qg5_sb = pers.tile([P, P], bf16, tag="qg5")
            v_sb = pers.tile([P, TT, 2 * HPC, DH + 1], bf16, tag="vsb")
            vg_sb = pers.tile([3 * 2 * G, DH + 1], bf16, tag="vg")
            ctx01_sb = pers.tile([P, S], bf16, tag="ctx01")
            ctx2_sb = pers.tile([DH + 1, S], bf16, tag="ctx2")
            ctxh1_sb = pers.tile([DH, S], bf16, tag="ctxh1")
            pg_sb = pers.tile([P, TT, HPC * G], bf16, tag="pgsb")

            # ---------------- constant loads ----------------
            # token ids + bv go FIRST, on the scalar queue, so the gathers
            # and the bvb broadcast are not stuck behind the weight loads
            ids_sb = pers.tile([P, TT], i32, tag="ids")
            nc.scalar.dma_start(out=ids_sb[:], in_=ids.ap())
            nc.scalar.dma_start(out=bv1_sb[:], in_=bv.ap()[None, :])
            nc.scalar.dma_start(out=eye_sb[:], in_=eye.ap())
            nc.sync.dma_start(
                out=wqk_sb[:], in_=wqk.ap().rearrange("(kt p) c -> p kt c", p=P)
            )
            nc.sync.dma_start(
                out=bqk_sb[:], in_=bqk.ap().rearrange("(kt p) -> p kt", p=P)
            )
            nc.sync.dma_start(
                out=wv_sb[:], in_=wv.ap().rearrange("(kt p) c -> p kt c", p=P)
            )
            nc.sync.dma_start(out=wo0_sb[:], in_=wo0.ap())
            nc.sync.dma_start(out=wo1_sb[:], in_=wo1.ap())
            nc.sync.dma_start(out=msk_sb[:], in_=msk.ap())
            c1_sb = pers.tile([1, 2 * QW], f32, tag="c1t")
            nc.gpsimd.memset(warm_sb[:], 0.0)
            nc.gpsimd.memset(c1_sb[:], 2.0017324)
            nc.gpsimd.memset(ones_sb[:], 1.0)
            nc.gpsimd.memset(v_sb[:, :, :, DH], 1.0)
            nc.gpsimd.memset(ctx2_sb[DH : DH + 1, :], 1.0)

            # PE warmup: ~17us of dependency-free matmuls so the HAM clock
            # gate reaches 8/8 (2.4 GHz) before the first real matmul, and
            # stays there through the gather ramp.
            with tc.tile_pool(name="warm_ps", bufs=1, space="PSUM") as wps:
                wpt = wps.tile([P, 512], f32, tag="wp")
                NWARM = 40
                for i in range(NWARM):
                    nc.tensor.matmul(
                        out=wpt[:], lhsT=warm_sb[:, 0:P], rhs=warm_sb[:],
                        start=(i == 0), stop=(i == NWARM - 1),
                    )

            # broadcast v-bias across partitions via ones-matmul (one-time)
            with tc.tile_pool(name="bcast_ps", bufs=1, space="PSUM") as bps:
                bvp = bps.tile([P, 384], f32, tag="bvp")
                nc.tensor.matmul(
                    out=bvp[:], lhsT=ones_sb[0:1, :], rhs=bv1_sb[:],
                    start=True, stop=True,
                )
                nc.vector.tensor_copy(out=bvb_sb[:], in_=bvp[:])

            # ---------------- gather + xbar transpose + projections --------
            with tc.tile_pool(name="gather", bufs=12) as gp, \
                 tc.tile_pool(name="xtp", bufs=3) as xp, \
                 tc.tile_pool(name="tp_ps", bufs=2, space="PSUM") as tps, \
                 tc.tile_pool(name="proj_ps", bufs=6, space="PSUM") as pps:

                xT_t = [None] * NT
                x8_t = [None] * NT

                def alloc_group(n):
                    xT_t[n] = xp.tile([P, FT, 512], bf16, tag="xT", name=f"xT{n}")
                    x8_t[n] = xp.tile([P, FT, 512], f8, tag="x8", name=f"x8{n}")

                xg_t = {}

                def emit_gather(tt):
                    xg = gp.tile([P, D], bf16, tag="xg")
                    nc.gpsimd.indirect_dma_start(
                        out=xg[:],
                        out_offset=None,
                        in_=emb.ap(),
                        in_offset=bass.IndirectOffsetOnAxis(
                            ap=ids_sb[:, tt : tt + 1], axis=0
                        ),
                    )
                    xg_t[tt] = xg

                def emit_transpose(tt):
                    # PE transpose: [tok, feat] -> [feat%P, ft, tok].  (The
                    # xbar dma_start_transpose path is Tile-serialized against
                    # every SWDGE gather DMA — a ~2us handoff per link — so
                    # identity matmuls + a PSUM copy are much faster here.)
                    n, j = divmod(tt, 4)
                    xg = xg_t.pop(tt)
                    tp = tps.tile([P, FT * P], bf16, tag="tp")
                    for ft in range(FT):
                        nc.tensor.transpose(
                            out=tp[:, ft * P : (ft + 1) * P],
                            in_=xg[:, ft * P : (ft + 1) * P],
                            identity=eye_sb[:],
                        )
                    dst = xT_t[n][:, :, j * P : (j + 1) * P]
                    if tt % 2 == 0:
                        nc.vector.tensor_copy(out=dst, in_=tp[:])
                    else:
                        nc.scalar.copy(out=dst, in_=tp[:])

                def emit_cast(n):
                    # fp8 copy (scaled) for the q/k projections, whole group
                    nc.vector.tensor_scalar_mul(
                        out=x8_t[n][:], in0=xT_t[n][:], scalar1=X8
                    )

                # q/k/qg/kg projections in fp8 DoubleRow (feature-major out).
                # block cols: 0:(q0|q1) 1:(k0|k1) 2:(q2|qg2) 3:(k2|kg2)
                #             4:(kg0|kg1); B5 (qg0|qg1) over token-tile 0 only
                desc = [DESC_Q, DESC_K, DESC_Q, DESC_K, DESC_K, DESC_Q]
                for n in range(3):
                    alloc_group(n)
                for tt in range(12):
                    emit_gather(tt)
                for tt in range(4):
                    emit_transpose(tt)
                emit_cast(0)
                for n in range(NT):
                    if n + 1 < NT:
                        for tt in range(4 * n + 4, 4 * n + 8):
                            emit_transpose(tt)
                        emit_cast(n + 1)
                    for bi in range(5):
                        ps = pps.tile([P, 512], f32, tag="pps")
                        for kt in range(FT // 2):
                            nc.tensor.matmul(
                                out=ps[:],
                                lhsT=wqk_sb[:, 2 * kt : 2 * kt + 2,
                                            bi * P : (bi + 1) * P],
                                rhs=x8_t[n][:, 2 * kt : 2 * kt + 2, :],
                                start=(kt == 0),
                                stop=(kt == FT // 2 - 1),
                                perf_mode=PM.DoubleRow,
                            )
                        nc.scalar.activation(
                            blk_sb[:, bi, n * 512 : (n + 1) * 512],
                            ps[:],
                            AF.Identity,
                            bias=bqk_sb[:, bi : bi + 1],
                            scale=desc[bi],
                        )
                    if n == 0:
                        # B5 (qg0|qg1) over token-tile 0 — must run while
                        # x8_t[0]'s ring slot is still live
                        ps5 = pps.tile([P, 512], f32, tag="pps")
                        for kt in range(FT // 2):
                            nc.tensor.matmul(
                                out=ps5[:, 0:P],
                                lhsT=wqk_sb[:, 2 * kt : 2 * kt + 2,
                                            5 * P : 6 * P],
                                rhs=x8_t[0][:, 2 * kt : 2 * kt + 2, 0:P],
                                start=(kt == 0),
                                stop=(kt == FT // 2 - 1),
                                perf_mode=PM.DoubleRow,
                            )
                        nc.scalar.activation(
                            qg5_sb[:], ps5[:, 0:P], AF.Identity,
                            bias=bqk_sb[:, 5:6], scale=desc[5],
                        )
                    # v/vg projections for the same token tiles (token-major)
                    for j in range(4):
                        tt = 4 * n + j
                        vp = pps.tile([P, 512], f32, tag="pps")
                        for kt in range(FT):
                            nc.tensor.matmul(
                                out=vp[:, 0:384],
                                lhsT=xT_t[n][:, kt, j * P : (j + 1) * P],
                                rhs=wv_sb[:, kt, :],
                                start=(kt == 0),
                                stop=(kt == FT - 1),
                            )
                        nc.vector.tensor_tensor(
                            out=v_sb[:, tt, :, 0:DH],
                            in0=vp[:, 0:384],
                            in1=bvb_sb[:],
                            op=OP.add,
                        )
                    # prefetch group n+3's gathers (after group n's readers
                    # are emitted, so the xT/x8 ring reuse deps are correct)
                    if n + 3 < NT:
                        alloc_group(n + 3)
                        for tt in range(4 * n + 12, 4 * n + 16):
                            emit_gather(tt)
                # global-token v rows (first G tokens), per head, incl ones
                # col, placed at partitions 32h to match the gk score tiles
                for h in range(HPC):
                    nc.sync.dma_start(
                        out=vg_sb[32 * h : 32 * h + G, :],
                        in_=v_sb[0:G, 0, h, :],
                    )

            # operand views (each matmul operand pair shares a base partition)
            qv = [blk_sb[0:DH, 0, :], blk_sb[DH:P, 0, :], blk_sb[0:DH, 2, :]]
            kv = [blk_sb[0:DH, 1, :], blk_sb[DH:P, 1, :], blk_sb[0:DH, 3, :]]
            qgv = [qg5_sb[0:DH, 0:G], qg5_sb[DH:P, 0:G], blk_sb[DH:P, 2, 0:G]]
            kgv = [blk_sb[0:DH, 4, :], blk_sb[DH:P, 4, :], blk_sb[DH:P, 3, :]]
            ctxdst = [ctx01_sb[0:DH, :], ctxh1_sb[:, :], ctx2_sb[0:DH, :]]

            # ---------------- attention ----------------
            with tc.tile_pool(name="pa_sb", bufs=4) as pap, \
                 tc.tile_pool(name="pb_sb", bufs=3) as pbp, \
                 tc.tile_pool(name="pgk_sb", bufs=3) as pgkp, \
                 tc.tile_pool(name="rec_sb", bufs=3) as recp, \
                 tc.tile_pool(name="sa_ps", bufs=2, space="PSUM") as spa, \
                 tc.tile_pool(name="sb_ps", bufs=1, space="PSUM") as spb, \
                 tc.tile_pool(name="gk_ps", bufs=1, space="PSUM") as spg, \
                 tc.tile_pool(name="ctx_ps", bufs=2, space="PSUM") as cps:

                gkf = spg.tile([3 * 2 * G, 2 * QW], f32, tag="gk")
                gk_ps = gkf[:]
                nc.vector.memset(gk_ps[:], 0.0)

                def emit_filler():
                    pass

                def emit_pv(h, c, pgk_t, ptiles, cpsum):
                    """PV accumulation for chunk c into its half of cpsum."""
                    cq = (c % 2) * QW
                    nc.tensor.matmul(
                        out=cpsum[:, cq : cq + QW],
                        lhsT=vg_sb[32 * h : 32 * h + G, :],
                        rhs=pgk_t[32 * h : 32 * h + G, cq : cq + QW],
                        start=True,
                        stop=False,
                        tile_position=(32 * h, 0),
                    )
                    for idx, (g_tt, p_ap, x0, x1) in enumerate(ptiles):
                        nc.tensor.matmul(
                            out=cpsum[:, cq + x0 : cq + x1],
                            lhsT=v_sb[:, g_tt, h, :],
                            rhs=p_ap,
                            start=False,
                            stop=(idx == len(ptiles) - 1),
                        )

                def emit_norm(st):
                    h, cp, cpsum, recb = st
                    nc.vector.tensor_tensor(
                        out=ctxdst[h][:, cp * 2 * QW : (cp + 1) * 2 * QW],
                        in0=cpsum[0:DH, :],
                        in1=recb[:],
                        op=OP.mult,
                    )

                def emit_scores(h, c, psb, cb):
                    a_slots, a_hi, a_mask, b_slots, b_rng, b_mask = \
                        _chunk_slots(c)
                    q0 = c * QW
                    psa = spa.tile([P, 1024], f32, tag="psa")
                    for jt, so in a_slots:
                        tok0 = (c - 1) * 2 * P + jt * P
                        nc.tensor.matmul(
                            out=psa[:, so : so + QW],
                            lhsT=kv[h][:, tok0 : tok0 + P],
                            rhs=qv[h][:, q0 : q0 + QW],
                            start=True, stop=True,
                        )
                    for jt, so in b_slots:
                        tok0 = (c - 1) * 2 * P + jt * P
                        nc.tensor.matmul(
                            out=psb[:, cb + so : cb + so + P],
                            lhsT=kv[h][:, tok0 : tok0 + P],
                            rhs=qv[h][:, q0 + so : q0 + so + P],
                            start=True, stop=True,
                        )
                    pA = pap.tile([P, 1024], bf16, tag="pa")
                    nc.scalar.activation(pA[:, 0:a_hi], psa[:, 0:a_hi], AF.Exp)
                    pB = pbp.tile([P, QW], bf16, tag="pb")
                    b0, b1 = b_rng
                    nc.scalar.activation(
                        pB[:, b0:b1], psb[:, cb + b0 : cb + b1], AF.Exp
                    )
                    for mo, mw, mso in (a_mask,):
                        nc.vector.tensor_tensor(
                            out=pA[:, mso : mso + mw],
                            in0=pA[:, mso : mso + mw],
                            in1=msk_sb[:, mo : mo + mw],
                            op=OP.mult,
                        )
                    mo, mw, mso = b_mask
                    nc.vector.tensor_tensor(
                        out=pB[:, mso : mso + mw],
                        in0=pB[:, mso : mso + mw],
                        in1=msk_sb[:, mo : mo + mw],
                        op=OP.mult,
                    )
                    ptiles = [
                        (2 * (c - 1) + jt, pA[:, so : so + QW], 0, QW)
                        for jt, so in a_slots
                    ] + [
                        (2 * (c - 1) + jt, pB[:, so : so + P], so, so + P)
                        for jt, so in b_slots
                    ]
                    return ptiles

                def emit_global_rows(h):
                    # global query rows for head h; independent of the
                    # windowed chain, interleaved to keep the PE duty high
                    gp_ps = spa.tile([P, 1024], f32, tag="psa")
                    for tt in range(TT):
                        nc.tensor.matmul(
                            out=gp_ps[:, tt * G : (tt + 1) * G],
                            lhsT=kgv[h][:, tt * P : (tt + 1) * P],
                            rhs=qgv[h][:],
                            start=True,
                            stop=True,
                        )
                    nc.scalar.activation(
                        pg_sb[:, :, h * G : (h + 1) * G],
                        gp_ps[:, 0 : TT * G],
                        AF.Exp,
                    )
                    gc_ps = cps.tile([DH + 1, 2 * QW], f32, tag="cps")
                    for tt in range(TT):
                        nc.tensor.matmul(
                            out=gc_ps[:, 0:G],
                            lhsT=v_sb[:, tt, HPC + h, :],
                            rhs=pg_sb[:, tt, h * G : (h + 1) * G],
                            start=(tt == 0),
                            stop=(tt == TT - 1),
                        )
                    deng = recp.tile([1, 2 * QW], f32, tag="den", name="dg")
                    nc.scalar.activation(
                        deng[:, 0:G], gc_ps[DH : DH + 1, 0:G], AF.Ln
                    )
                    recg = recp.tile([1, 2 * QW], bf16, tag="rec")
                    nc.scalar.activation(
                        recg[:, 0:G], deng[:, 0:G], AF.Exp, scale=-1.0
                    )
                    recgb = recp.tile([DH, 2 * QW], bf16, tag="recb")
                    nc.sync.dma_start(
                        out=recgb[:, None, 0:G],
                        in_=recg[0:1, None, 0:G].to_broadcast([1, DH, G]),
                    )
                    nc.vector.tensor_tensor(
                        out=ctxdst[h][:, 0:G],
                        in0=gc_ps[0:DH, 0:G],
                        in1=recgb[:, 0:G],
                        op=OP.mult,
                    )

                from collections import deque

                pending = deque()    # (h, c, pgk, ptiles, cpsum); PV lag 2

                def emit_recip(h, cp, cpsum):
                    """Paired-chunk denominator -> 1/x -> broadcast DMA."""
                    ln_t = recp.tile([1, 2 * QW], f32, tag="den", name="lnt")
                    nc.scalar.activation(
                        ln_t[:], cpsum[DH : DH + 1, :], AF.Ln
                    )
                    rec = recp.tile([1, 2 * QW], bf16, tag="rec")
                    nc.scalar.activation(rec[:], ln_t[:], AF.Exp, scale=-1.0)
                    recb = recp.tile([DH, 2 * QW], bf16, tag="recb")
                    nc.sync.dma_start(
                        out=recb[:, None, :],
                        in_=rec[0:1, None, :].to_broadcast([1, DH, 2 * QW]),
                    )
                    return (h, cp, cpsum, recb)

                norm_box = [None]

                def pop_pv():
                    ent = pending.popleft()
                    emit_pv(*ent)
                    emit_filler()
                    ph, pc = ent[0], ent[1]
                    if pc % 2 == 1:
                        st = emit_recip(ph, pc // 2, ent[4])
                        if norm_box[0] is not None:
                            emit_norm(norm_box[0])
                        norm_box[0] = st

                def emit_gk(cp):
                    # global-key scores + exp for pair cp (prefetched one
                    # pair early so the exp isn't part of the pair-boundary
                    # ACT bundle that stalls the PE)
                    q0 = cp * 2 * QW
                    for h in range(HPC):
                        nc.tensor.matmul(
                            out=gk_ps[32 * h : 32 * h + G, :],
                            lhsT=kv[h][:, 0:G],
                            rhs=qv[h][:, q0 : q0 + 2 * QW],
                            start=True, stop=True,
                            tile_position=(0 if h != 1 else DH, 32 * h),
                        )
                    pgk_t = pgkp.tile([3 * 2 * G, 2 * QW], bf16, tag="pgk")
                    nc.scalar.activation(pgk_t[:], gk_ps[:], AF.Exp)
                    return pgk_t

                pgk_next = emit_gk(0)
                for cp in range(C // 2):
                    pgk_t = pgk_next
                    for h in range(HPC):
                        cpsum = cps.tile([DH + 1, 2 * QW], f32, tag="cps")
                        psb = spb.tile([P, 2 * QW], f32, tag="psb")
                        for ci, c in enumerate((2 * cp, 2 * cp + 1)):
                            ptiles = emit_scores(h, c, psb, ci * QW)
                            emit_filler()
                            pending.append((h, c, pgk_t, ptiles, cpsum))
                            if len(pending) > 2:
                                pop_pv()
                        if h == 0 and cp + 1 < C // 2:
                            pgk_next = emit_gk(cp + 1)
                while pending:
                    pop_pv()
                emit_norm(norm_box[0])

            # ------- output projection, interleaved with global rows -------
            # (bias via ones row; global rows fill the PE queue's latency
            # bubbles between out-proj tiles and vice versa)
            with tc.tile_pool(name="out_sb", bufs=4) as osb, \
                 tc.tile_pool(name="out_ps", bufs=3, space="PSUM") as ops:

                def emit_outproj(tt):
                    op_ps = ops.tile([P, D], f32, tag="ops", name="opps")
                    # group by lhsT so the PE loads each weight set once
                    for (n0, n1) in ((0, 512), (512, 768)):
                        nc.tensor.matmul(
                            out=op_ps[:, n0:n1],
                            lhsT=ctx01_sb[:, tt * P : (tt + 1) * P],
                            rhs=wo0_sb[:, n0:n1],
                            start=True,
                            stop=False,
                        )
                    for (n0, n1) in ((0, 512), (512, 768)):
                        nc.tensor.matmul(
                            out=op_ps[:, n0:n1],
                            lhsT=ctx2_sb[:, tt * P : (tt + 1) * P],
                            rhs=wo1_sb[:, n0:n1],
                            start=False,
                            stop=True,
                        )
                    ot = osb.tile([P, D], bf16, tag="ot")
                    if tt % 2 == 0:
                        nc.vector.tensor_copy(out=ot[:], in_=op_ps[:])
                    else:
                        nc.scalar.copy(out=ot[:], in_=op_ps[:])
                    nc.sync.dma_start(
                        out=outp.ap()[tt * P : (tt + 1) * P, :], in_=ot[:]
                    )

                def emit_grows(h):
                    # global query rows for head h (own 1-bank psum tags)
                    gp_ps = ops.tile(
                        [P, 2 * QW], f32, tag="gps", bufs=1, name="gpps"
                    )
                    for tt in range(TT):
                        nc.tensor.matmul(
                            out=gp_ps[:, tt * G : (tt + 1) * G],
                            lhsT=kgv[h][:, tt * P : (tt + 1) * P],
                            rhs=qgv[h][:],
                            start=True,
                            stop=True,
                        )
                    nc.scalar.activation(
                        pg_sb[:, :, h * G : (h + 1) * G],
                        gp_ps[:, 0 : TT * G],
                        AF.Exp,
                    )
                    gc_ps = ops.tile(
                        [DH + 1, 2 * QW], f32, tag="gcs", bufs=1, name="gcps"
                    )
                    for tt in range(TT):
                        nc.tensor.matmul(
                            out=gc_ps[:, 0:G],
                            lhsT=v_sb[:, tt, HPC + h, :],
                            rhs=pg_sb[:, tt, h * G : (h + 1) * G],
                            start=(tt == 0),
                            stop=(tt == TT - 1),
                        )
                    deng = osb.tile([1, 2 * QW], f32, tag="gden", name="gdn")
                    nc.scalar.activation(
                        deng[:, 0:G], gc_ps[DH : DH + 1, 0:G], AF.Ln
                    )
                    recg = osb.tile([1, 2 * QW], bf16, tag="grec", name="grc")
                    nc.scalar.activation(
                        recg[:, 0:G], deng[:, 0:G], AF.Exp, scale=-1.0
                    )
                    recgb = osb.tile([DH, 2 * QW], bf16, tag="grcb", name="grb")
                    nc.sync.dma_start(
                        out=recgb[:, None, 0:G],
                        in_=recg[0:1, None, 0:G].to_broadcast([1, DH, G]),
                    )
                    nc.vector.tensor_tensor(
                        out=ctxdst[h][:, 0:G],
                        in0=gc_ps[0:DH, 0:G],
                        in1=recgb[:, 0:G],
                        op=OP.mult,
                    )

                # head 1 ctx tokens P:S -> ctx01 rows 64:128 (cols 0:P wait
                # for global head 1, which overwrites its gctx columns)
                nc.sync.dma_start(
                    out=ctx01_sb[DH:P, P:S], in_=ctxh1_sb[:, P:S]
                )
                emit_grows(0)
                for tt in range(1, 12):
                    emit_outproj(tt)
                emit_grows(1)
                nc.sync.dma_start(
                    out=ctx01_sb[DH:P, 0:P], in_=ctxh1_sb[:, 0:P]
                )
                for tt in range(12, 22):
                    emit_outproj(tt)
                emit_grows(2)
                for tt in range(22, TT):
                    emit_outproj(tt)
                emit_outproj(0)

    # populate .instr bytes for extended-inst InstISA subclasses (the custom
    # DVE reciprocal) — raw Bass skips this pass and the NEFF compiler then
    # fails with "ISA wrong length"
    from concourse import library_overlay

    library_overlay.lower_extended_insts(nc)
    return nc


def _prep_core_inputs(core, input_ids, emb, Wq, bq, Wk, bk, Wv, bv,
                      Wqg, bqg, Wkg, bkg, Wvg, bvg, Wo, bo, _emb_bf=None):
    b, hg = divmod(core, 4)
    hs = HPC * hg * DH           # feature offset of this core's head slice
    sl = slice(hs, hs + HPC * DH)

    def hcol(Wm, h):
        return np.asarray(Wm[:, hs + h * DH : hs + (h + 1) * DH], np.float32)

    def hbias(bm, h):
        return np.asarray(bm[hs + h * DH : hs + (h + 1) * DH], np.float32)

    # blocks: 0:(q0|q1) 1:(k0|k1) 2:(q2|qg2) 3:(k2|kg2) 4:(kg0|kg1) 5:(qg0|qg1)
    wq = [hcol(Wq, h) for h in range(HPC)]
    wk = [hcol(Wk, h) for h in range(HPC)]
    wqg = [hcol(Wqg, h) for h in range(HPC)]
    wkg = [hcol(Wkg, h) for h in range(HPC)]
    bq_ = [hbias(bq, h) * SCALE for h in range(HPC)]
    bk_ = [hbias(bk, h) for h in range(HPC)]
    bqg_ = [hbias(bqg, h) * SCALE for h in range(HPC)]
    bkg_ = [hbias(bkg, h) for h in range(HPC)]

    wqk_cat = np.concatenate(
        [wq[0], wq[1], wk[0], wk[1], wq[2], wqg[2], wk[2], wkg[2],
         wkg[0], wkg[1], wqg[0], wqg[1]], axis=1) * X8
    bqk_cat = np.concatenate(
        [bq_[0], bq_[1], bk_[0], bk_[1], bq_[2], bqg_[2], bk_[2], bkg_[2],
         bkg_[0], bkg_[1], bqg_[0], bqg_[1]])

    wv_cat = np.concatenate(
        [hcol(Wv, h) for h in range(HPC)] + [hcol(Wvg, h) for h in range(HPC)],
        axis=1)
    bv_cat = np.concatenate(
        [hbias(bv, h) for h in range(HPC)] + [hbias(bvg, h) for h in range(HPC)])

    wo_cat = np.asarray(Wo[sl, :], np.float32)
    bo_row = (np.asarray(bo, np.float32) if hg == 0
              else np.zeros((D,), np.float32))
    wo1_cat = np.concatenate([wo_cat[P : P + DH, :], bo_row[None, :]], axis=0)

    if _emb_bf is None:
        _emb_bf = np.ascontiguousarray(np.asarray(emb, np.float32)).astype(BF16)

    return {
        "ids": np.ascontiguousarray(
            np.asarray(input_ids[b], np.int32).reshape(TT, P).T),
        "emb": _emb_bf,
        "wqk": wqk_cat.astype(F8E4),
        "bqk": bqk_cat.astype(np.float32),
        "wv": wv_cat.astype(BF16),
        "bv": bv_cat.astype(np.float32),
        "wo0": np.ascontiguousarray(wo_cat[0:P, :]).astype(BF16),
        "wo1": np.ascontiguousarray(wo1_cat).astype(BF16),
        "msk": _build_masks(),
        "eye": np.eye(P, dtype=np.float32).astype(BF16),
    }


def kernel(**inputs):
    _install_axon_hooks()
    from concourse.bass_utils import run_bass_kernel_spmd

    if "nc" not in _COMPILED:
        _COMPILED["nc"] = build_nc()
    nc = _COMPILED["nc"]

    emb_bf = np.ascontiguousarray(
        np.asarray(inputs["emb"], np.float32)).astype(BF16)
    in_maps = [
        _prep_core_inputs(core, _emb_bf=emb_bf, **inputs)
        for core in range(NCORES)
    ]
    trace = bool(int(os.environ.get("KERNEL_TRACE", "0")))
    res = run_bass_kernel_spmd(nc, in_maps, list(range(NCORES)), trace=trace)
    _COMPILED["last_result"] = res

    out = np.zeros((B, S, D), np.float32)
    for core in range(NCORES):
        out[core // 4] += np.asarray(res.results[core]["out"], np.float32)
    return out
